# revision 6
# baseline (speedup 1.0000x reference)
"""DUQ RBF head kernel for Trainium2 (8 NeuronCores, batch-parallel).

Computes out[b,c,h,w] = exp(gamma * mean_e (einsum('bfhw,ecf', x, W) - m/N)^2)
for features [8,512,128,128], weights [16,64,512], m [16,64], N [64].

Strategy: data-parallel over batch (1 image per core). Per core, one big
matmul [ec=1024, f=512] @ [f=512, pix=16384] on the tensor engine.

v5: both matmul operands fp16 (exact products, fp32 PSUM accumulate;
quantization error ~2e-3 << 2e-2 tolerance). fp16 halves feature DMA bytes
and makes LDWEIGHTS fast (FWL) so the matmul stream runs at its 216 ns
issue-gap floor. Tile order 512,512,(7x2048),512,512: the leading small
tiles start compute ~7us earlier (first matmul only needs 0.5 MB of
features + the m=0 weight chunk, loaded as its own small DMA), the middle
2048-wide super-tiles amortize ACT/DVE fixed overheads, and the trailing
small tiles shrink the serial post-matmul tail. Small tiles stripe their
8 matmul groups across the 4 banks of a 2048-wide PSUM slot so the scalar
engine never blocks the PE. Features ride the Sync HWDGE queue; weights
(m=0 chunk first, rest as one big-descriptor DMA) + centroid bias ride
the Scalar HWDGE queue in parallel.
"""

import numpy as np

import concourse.bacc as bacc_mod
import concourse.mybir as mybir
import concourse.tile as tile
from concourse.bass_utils import run_bass_kernel_spmd

dt = mybir.dt
Act = mybir.ActivationFunctionType

B, F, H, W = 8, 512, 128, 128
E, C = 16, 64
PIX = H * W           # 16384 pixels per image
ST = 2048             # super-tile width (4 psum banks)
MCH = (E * C) // 128  # 8 ec-chunks of 128 partitions
KCH = F // 128        # 4 contraction chunks
LENGTH_SCALE = 0.1
GAMMA = -1.0 / (2.0 * LENGTH_SCALE**2)   # -50.0
EXP_SCALE = GAMMA / E                    # -3.125

# (start_px, width) processing order: small, small, 7 super, small, small
TILES = (
    [(0, 512), (512, 512)]
    + [(1024 + t * ST, ST) for t in range(7)]
    + [(15360, 512), (15872, 512)]
)
assert sum(w for _, w in TILES) == PIX


def _build():
    nc = bacc_mod.Bacc(None)
    feat_d = nc.declare_dram_parameter("feat", [F, PIX], dt.float16, isOutput=False)
    wt_d = nc.declare_dram_parameter("wt", [F, E * C], dt.float16, isOutput=False)
    negc_d = nc.declare_dram_parameter("negc", [128, MCH], dt.float32, isOutput=False)
    out_d = nc.declare_dram_parameter("out", [C, PIX], dt.float32, isOutput=True)

    feat_k = feat_d.rearrange("(k p) x -> p k x", k=KCH)
    wt_k = wt_d.rearrange("(k p) m -> p k m", k=KCH)

    with tile.TileContext(nc) as tc:
        with (
            tc.tile_pool(name="singles", bufs=1) as singles,
            tc.tile_pool(name="xin", bufs=3) as xin,
            tc.tile_pool(name="sqp", bufs=3) as sqp,
            tc.tile_pool(name="accp", bufs=2) as accp,
            tc.tile_pool(name="outp", bufs=2) as outp,
            tc.tile_pool(name="ps", bufs=2, space="PSUM") as ps,
        ):
            # Scalar HWDGE ring: m=0 weight chunk first (gates the very
            # first matmul), then the centroid bias, then the remaining
            # seven chunks as one big-descriptor DMA.
            ws0 = singles.tile([128, KCH, 128], dt.float16, tag="ws0")
            nc.scalar.dma_start(out=ws0, in_=wt_k[:, :, 0:128])
            negc_sb = singles.tile([128, MCH], dt.float32, tag="negc")
            nc.scalar.dma_start(out=negc_sb, in_=negc_d[:, :])
            wsr = singles.tile([128, KCH, (MCH - 1) * 128], dt.float16, tag="wsr")
            nc.scalar.dma_start(out=wsr, in_=wt_k[:, :, 128:])

            def lhs(m, k):
                if m == 0:
                    return ws0[:, k, :]
                return wsr[:, k, (m - 1) * 128 : m * 128]

            # Sync HWDGE ring: feature tiles in processing order.
            xtiles = []
            for px0, width in TILES:
                xt = []
                for k in range(KCH):
                    xtk = xin.tile([128, width], dt.float16, tag=f"x{k}")
                    nc.sync.dma_start(
                        out=xtk, in_=feat_k[:, k, px0 : px0 + width]
                    )
                    xt.append(xtk)
                xtiles.append(xt)

            for (px0, width), xt in zip(TILES, xtiles):
                acc = accp.tile([128, width], dt.float32, tag="acc")
                pst = None
                for m in range(MCH):
                    if width == ST:
                        pst = ps.tile([128, ST], dt.float32, tag="mm")
                        view = pst
                        nsl = ST // 512
                    else:
                        # stripe small-tile groups across the 4 banks of a
                        # 2048-wide psum slot: 8 banks stay in flight
                        if m % 4 == 0:
                            pst = ps.tile([128, ST], dt.float32, tag="mm")
                        view = pst[:, (m % 4) * 512 : (m % 4 + 1) * 512]
                        nsl = 1
                    for k in range(KCH):
                        for s in range(nsl):
                            sl = slice(s * 512, (s + 1) * 512)
                            nc.tensor.matmul(
                                out=view[:, sl], lhsT=lhs(m, k),
                                rhs=xt[k][:, sl],
                                start=(k == 0), stop=(k == KCH - 1),
                            )
                    if m == 0:
                        nc.scalar.activation(
                            out=acc, in_=view, func=Act.Square,
                            bias=negc_sb[:, 0:1], scale=1.0,
                        )
                    else:
                        sq = sqp.tile([128, width], dt.float32, tag="sq")
                        nc.scalar.activation(
                            out=sq, in_=view, func=Act.Square,
                            bias=negc_sb[:, m : m + 1], scale=1.0,
                        )
                        nc.vector.tensor_add(out=acc, in0=acc, in1=sq)

                tmp = outp.tile([64, width], dt.float32, tag="tmp")
                nc.vector.tensor_copy(out=tmp, in_=acc[64:128, :])
                hc = outp.tile([64, width], dt.float32, tag="hc")
                nc.vector.tensor_add(out=hc, in0=acc[0:64, :], in1=tmp)
                eo = outp.tile([64, width], dt.float32, tag="eo")
                nc.scalar.activation(
                    out=eo, in_=hc, func=Act.Exp, bias=0.0, scale=EXP_SCALE
                )
                nc.scalar.dma_start(out=out_d[:, px0 : px0 + width], in_=eo)

    nc.finalize()
    return nc


_NC_CACHE = {}


def _get_nc():
    if "nc" not in _NC_CACHE:
        _NC_CACHE["nc"] = _build()
    return _NC_CACHE["nc"]


def _prep_inputs(features, weights, m, N):
    # wt[f, e*64+c] = weights[e, c, f]
    wt = np.ascontiguousarray(
        weights.astype(np.float32).transpose(2, 0, 1).reshape(F, E * C)
    ).astype(np.float16)
    cent = (m.astype(np.float32) / N.astype(np.float32)[None, :]).reshape(-1)  # [ec]
    negc = np.ascontiguousarray(-cent.reshape(MCH, 128).T)  # [128, MCH]
    feats = np.ascontiguousarray(
        features.astype(np.float16).reshape(B, F, PIX)
    )
    return [{"feat": feats[i], "wt": wt, "negc": negc} for i in range(B)]


def run_spmd(features, weights, m, N, trace=False):
    in_maps = _prep_inputs(features, weights, m, N)
    res = run_bass_kernel_spmd(_get_nc(), in_maps, list(range(B)), trace=trace)
    out = np.stack([res.results[i]["out"] for i in range(B)])  # [B, C, PIX]
    return out.reshape(B, C, H, W).astype(np.float32), res


def kernel(features, weights, m, N):
    out, _ = run_spmd(features, weights, m, N, trace=False)
    return out
